# revision 7
# baseline (speedup 1.0000x reference)
"""Trainium2 Bass kernel for nn_MhsLayer (biaffine pairwise logits).

Math:
  u = x @ Wu + bu ; v = x @ Wv + bv
  pu = u @ Wuv[:in] ; pv = v @ Wuv[in:]
  logits[b,r,i,j] = pu[b,j,r] + pv[b,i,r], masked to NEG where mask[i]==0 or mask[j]==0

Sharding: data-parallel over batch, one batch element per NeuronCore (8 cores).

v3 design (fp16 output, halved HBM write traffic vs the f32 baseline):
  Host folds the linear chain into Af = [Wu@Wuv[:in] | Wv@Wuv[in:]] (256x8)
  and cf = [cu; cv] (8,), ships x pre-transposed in fp16 with Af appended.

  Device per core:
    1. x DMAs issued first; PE warmup matmuls open the HAM clock gate.
    2. puv = Af^T @ xT -> PSUM [8,1024]; one DVE scalar_tensor_tensor
       -> puvm = (puv+cf)*mask  (rows 0-3 = masked pu+cu, 4-7 = masked pv+cv).
    3. puvm rows flatten into 2-partition cat operands next to mask rows.
    4. Bulk: 64 rank-2 fp16 matmuls  out = pvm_i*m_j + m_i*pum_j  -> PSUM
       (masked entries come out 0 vs reference -1e-12; error 1e-12).
       PSUM -> SBUF fp16 drains split across ACT and DVE; two tiles pair
       into one [128,2048] obuf flushed as a single 512KB DMA, alternating
       the two HWDGE queues (8 MiB per core; ~405 GB/s sustainable).
  Host converts fp16 -> f32; fp16 rounding gives rel err ~8e-4 (gate 2e-2).
"""

import sys

import numpy as np

if "/opt/trn_rl_repo" not in sys.path:
    sys.path.insert(0, "/opt/trn_rl_repo")

B, L, IN, OUT = 8, 1024, 256, 4
N_CORES = 8
NT = L // 128  # 8 token tiles
XC = L + 4 * OUT  # x0 cols: 1024 x | 8 Af0 | 8 Af1
ACT_UNITS = 9  # of every 16 drain units, this many go to ACT (rest DVE)


def build_nc():
    """Build the per-core Bass program (SPMD: same program, per-core inputs)."""
    import concourse.bass as bass
    import concourse.tile as tile
    from concourse import bacc, mybir

    f32 = mybir.dt.float32
    fp16 = mybir.dt.float16
    Alu = mybir.AluOpType

    nc = bacc.Bacc("TRN2", target_bir_lowering=False, debug=False, num_devices=1)

    x0_d = nc.dram_tensor("x0", (IN // 2, XC), fp16, kind="ExternalInput").ap()
    x1_d = nc.dram_tensor("x1", (IN // 2, L), fp16, kind="ExternalInput").ap()
    m8_d = nc.dram_tensor("m8", (2 * OUT, L + 1), f32, kind="ExternalInput").ap()
    mb4_d = nc.dram_tensor("mb4", (1, OUT * L), fp16, kind="ExternalInput").ap()
    out_d = nc.dram_tensor("out", (OUT, L, L), fp16, kind="ExternalOutput").ap()

    with tile.TileContext(nc) as tc:
        with (
            tc.tile_pool(name="const", bufs=1) as const_pool,
            tc.tile_pool(name="xt", bufs=1) as xt_pool,
            tc.tile_pool(name="small", bufs=1) as small_pool,
            tc.tile_pool(name="obuf", bufs=16) as obuf_pool,
        ):
            # ---- input DMAs first: x halves on the two HWDGE queues
            x0t = xt_pool.tile([128, XC], fp16, tag="x0t")
            nc.sync.dma_start(x0t[:], x0_d)
            x1t = xt_pool.tile([128, L], fp16, tag="x1t")
            nc.scalar.dma_start(x1t[:], x1_d)
            m8t = const_pool.tile([2 * OUT, L + 1], f32, tag="m8t")
            nc.gpsimd.dma_start(m8t[:], m8_d)

            m8 = m8t[:, 0:L]
            cf_sb = m8t[:, L : L + 1]
            af0 = x0t[:, L : L + 2 * OUT]
            af1 = x0t[:, L + 2 * OUT : L + 4 * OUT]

            # cat operands: 2 partitions each.
            # lhs_cat: p0 = pvm rows (r-blocks), p1 = mask row x4
            # rhs_cat: p0 = mask row x4,        p1 = pum rows
            lhs_cat = small_pool.tile([2, OUT * L], fp16, tag="lhs_cat")
            rhs_cat = small_pool.tile([2, OUT * L], fp16, tag="rhs_cat")
            nc.sync.dma_start(rhs_cat[0:1, :], mb4_d)
            nc.scalar.dma_start(lhs_cat[1:2, :], mb4_d)

            # ---- PE warmup: open the HAM clock gate while inputs land
            with tc.tile_pool(name="warm", bufs=1, space="PSUM") as warm_pool:
                wtile = const_pool.tile([128, 512], fp16, tag="wtile")
                nc.vector.memset(wtile[:], 0.0)
                wp = warm_pool.tile([128, 512], f32, tag="wp")
                for _ in range(16):
                    nc.tensor.matmul(wp[:], wtile[:, :128], wtile[:], start=True, stop=True)

            # ---- projection: puv rows in PSUM, masked+biased -> puvm
            puvm = small_pool.tile([2 * OUT, L], fp16, tag="puvm")
            with tc.tile_pool(name="ppsum", bufs=1, space="PSUM") as ppsum_pool:
                pp = ppsum_pool.tile([2 * OUT, L], f32, tag="pp")
                for jh in range(2):
                    sl = slice(jh * 512, (jh + 1) * 512)
                    nc.tensor.matmul(
                        pp[:, sl], af0, x0t[:, 0:L][:, sl], start=True, stop=False
                    )
                for jh in range(2):
                    sl = slice(jh * 512, (jh + 1) * 512)
                    nc.tensor.matmul(
                        pp[:, sl], af1, x1t[:, sl], start=False, stop=True
                    )
                nc.vector.scalar_tensor_tensor(
                    puvm[:], pp[:], cf_sb, m8, Alu.add, Alu.mult
                )
                # flatten pum/pvm rows into the cat operands (single DMAs)
                nc.sync.dma_start(
                    rhs_cat[1:2, :].rearrange("p (r t) -> p r t", r=OUT),
                    puvm[0:OUT, :],
                )
                nc.scalar.dma_start(
                    lhs_cat[0:1, :].rearrange("p (r t) -> p r t", r=OUT),
                    puvm[OUT : 2 * OUT, :],
                )

            # ---- bulk: 32 output tiles, 2 per obuf, drains split ACT/DVE ----
            with tc.tile_pool(name="bpsum", bufs=3, space="PSUM") as bpsum_pool:
                k = 0  # flush counter
                u = 0  # drain unit counter

                for r in range(OUT):
                    for n2 in range(NT // 2):
                        ob = obuf_pool.tile(
                            [128, 2 * L], fp16, tag="ob", name=f"ob_{r}_{n2}"
                        )
                        for tw in range(2):
                            n = 2 * n2 + tw
                            bp = bpsum_pool.tile(
                                [128, L], f32, tag="bp", name=f"bp_{r}_{n}"
                            )
                            for jh in range(2):
                                nc.tensor.matmul(
                                    bp[:, jh * 512 : (jh + 1) * 512],
                                    lhs_cat[
                                        :, r * L + n * 128 : r * L + (n + 1) * 128
                                    ],
                                    rhs_cat[
                                        :, r * L + jh * 512 : r * L + (jh + 1) * 512
                                    ],
                                    start=True,
                                    stop=True,
                                )
                            dst = ob[:, tw * L : (tw + 1) * L]
                            if u % 16 < ACT_UNITS:
                                nc.scalar.copy(dst, bp[:])
                            else:
                                nc.vector.tensor_copy(dst, bp[:])
                            u += 1
                        dstd = out_d[
                            r, n2 * 256 : (n2 + 1) * 256, :
                        ].rearrange("(two p) f -> p two f", two=2)
                        srcd = ob[:].rearrange("p (two f) -> p two f", two=2)
                        if k % 2 == 0:
                            nc.sync.dma_start(dstd, srcd)
                        else:
                            nc.scalar.dma_start(dstd, srcd)
                        k += 1

    nc.compile()
    return nc


_NC = None


def _get_nc():
    global _NC
    if _NC is None:
        _NC = build_nc()
    return _NC


def make_in_maps(inputs, mask, Wu, bu, Wv, bv, Wuv):
    Af = np.concatenate(
        [
            Wu.astype(np.float64) @ Wuv[:IN].astype(np.float64),
            Wv.astype(np.float64) @ Wuv[IN:].astype(np.float64),
        ],
        axis=1,
    )  # (256, 8) [Au | Av]
    cf = (
        np.concatenate(
            [
                bu.astype(np.float64) @ Wuv[:IN].astype(np.float64),
                bv.astype(np.float64) @ Wuv[IN:].astype(np.float64),
            ]
        )
        .astype(np.float32)
        .reshape(2 * OUT, 1)
    )
    in_maps = []
    for b in range(B):
        mf = mask[b].astype(np.float32).reshape(1, L)
        xT = inputs[b].T.astype(np.float16)
        x0 = np.zeros((IN // 2, XC), dtype=np.float16)
        x0[:, :L] = xT[: IN // 2]
        x0[:, L : L + 2 * OUT] = Af[: IN // 2].astype(np.float16)
        x0[:, L + 2 * OUT : L + 4 * OUT] = Af[IN // 2 :].astype(np.float16)
        m8 = np.concatenate(
            [np.broadcast_to(mf, (2 * OUT, L)), np.broadcast_to(cf, (2 * OUT, 1))],
            axis=1,
        )
        mb4 = np.tile(mf.astype(np.float16), (1, OUT))
        in_maps.append(
            {
                "x0": x0,
                "x1": np.ascontiguousarray(xT[IN // 2 :]),
                "m8": np.ascontiguousarray(m8, dtype=np.float32),
                "mb4": mb4,
            }
        )
    return in_maps


def kernel(inputs, mask, Wu, bu, Wv, bv, Wuv):
    from concourse import bass_utils

    inputs = np.asarray(inputs, dtype=np.float32)
    mask = np.asarray(mask)
    Wu = np.asarray(Wu, dtype=np.float32)
    bu = np.asarray(bu, dtype=np.float32)
    Wv = np.asarray(Wv, dtype=np.float32)
    bv = np.asarray(bv, dtype=np.float32)
    Wuv = np.asarray(Wuv, dtype=np.float32)
    nc = _get_nc()
    in_maps = make_in_maps(inputs, mask, Wu, bu, Wv, bv, Wuv)
    res = bass_utils.run_bass_kernel_spmd(nc, in_maps, core_ids=list(range(N_CORES)))
    out = np.stack([res.results[c]["out"] for c in range(N_CORES)], axis=0)
    return np.ascontiguousarray(out.astype(np.float32))
